# revision 22
# baseline (speedup 1.0000x reference)
"""Causal self-attention (B=4, T=2048, C=1024, H=16, Dh=64) on 8 TRN2 cores.

Sharding: tensor-parallel over heads. Core c owns heads {2c, 2c+1}:
  - w_qkv column-sliced -> [1024, 384] (128 q-cols | 128 k-cols | 128 v-cols)
  - w_proj row-sliced   -> [128, 1024]
  - x is broadcast to all cores, pre-transposed on host to feature-major
    x_t [1024, B*T] so the contraction dim (C) lands on SBUF partitions.
  - each core emits a partial y [B*T, 1024]; host sums the 8 partials and
    adds b_proj.

Per-core pipeline (all matmuls fp32r = full-rate fp32):
  A. QKV: qkv_T = w_slice.T @ x_t -> q,k feature-major [128, T] fp32r;
     v feature-major f32, then PE-transposed to token-major [Tk, 65] with a
     fused ones column (row 64 of the att@v psum = softmax denominator l).
  B. Attention per (head, 1024-query group), key chunks of 128:
     S^T = k_chunk.T @ q (K=64), causal mask add on the diagonal chunk,
     Exp on ACT (scale=1/8, no max subtraction - scores are O(10)),
     att@v accumulates y^T[65, Tq] in PSUM.
  C. Proj: y_part[tok, :] = y_T.T @ w_proj_slice, written token-major.

Scheduling notes. TRN2 engines execute their queues IN ORDER and the PE
clock halves (HAM) whenever the PE idles more than a few us, so emission
order is arranged to keep the PE stream dense:
  - av matmuls trail the S matmuls by one key chunk (never wait on ACT);
  - QKV is a resumable step stream; next-batch steps are interleaved as PE
    filler at attention group starts. x chunks are 256 tokens, DMA'd two
    chunks ahead, split across the sync+gpsimd queues, and rounded to
    fp32r on the (otherwise idle) GpSimd engine so the DVE queue stays
    clear;
  - softmax normalization is split: one DVE copy frees the psum
    accumulator, l bounces through DRAM into [128, 8] for a parallel
    reciprocal, and the PE tail (K=1 ones-matmul broadcast of r + DVE
    multiply into y_T) is deferred into the next group's kt loop.
"""

import sys

sys.path.insert(0, "/opt/trn_rl_repo")
import numpy as np

B = 4
T = 2048
C = 1024
H = 16
DH = 64
NCORES = 8
CPC = 128  # channels (=2 heads * 64) per core
TT = B * T
P = 128
XC = 256  # x-chunk tokens
NCH = T // XC  # 8 chunks per batch

TRACE = False
LAST_EXEC_NS = None

_NC_CACHE = []


def _build():
    import concourse.bacc as bacc
    import concourse.mybir as mybir
    import concourse.tile as tile
    from concourse.masks import make_identity

    f32 = mybir.dt.float32
    r32 = mybir.dt.float32r
    Exp = mybir.ActivationFunctionType.Exp
    ADD = mybir.AluOpType.add
    MULT = mybir.AluOpType.mult

    nc = bacc.Bacc(target_bir_lowering=False)

    x_t = nc.dram_tensor("x_t", [C, TT], f32, kind="ExternalInput")
    w_qkv_c = nc.dram_tensor("w_qkv_c", [C, 3 * CPC], f32, kind="ExternalInput")
    b_qkv_c = nc.dram_tensor("b_qkv_c", [P, 3], f32, kind="ExternalInput")
    w_proj_c = nc.dram_tensor("w_proj_c", [CPC, C], f32, kind="ExternalInput")
    y_part = nc.dram_tensor("y_part", [TT, C], f32, kind="ExternalOutput")

    xt_ap = x_t.rearrange("(ko p) t -> p ko t", p=P)  # [128, 8, TT]
    wq_ap = w_qkv_c.rearrange("(ko p) f -> p ko f", p=P)  # [128, 8, 384]

    def segs(off):
        """Split [off, 1024) at the 512 psum-bank boundary."""
        if off < 512:
            return [(off, 512), (512, 1024)]
        if off < 1024:
            return [(off, 1024)]
        return []

    with tile.TileContext(nc) as tc:
        with (
            tc.tile_pool(name="cst", bufs=1) as cst,
            tc.tile_pool(name="stage", bufs=3) as stage,
            tc.tile_pool(name="xr", bufs=3) as xrp,
            tc.tile_pool(name="qk", bufs=2) as qkp,
            tc.tile_pool(name="va", bufs=2) as vap,
            tc.tile_pool(name="yt", bufs=2) as ytp,
            tc.tile_pool(name="es", bufs=2) as esp,
            tc.tile_pool(name="oo", bufs=2) as oop,
            tc.tile_pool(name="rr", bufs=1) as rrp,
            tc.tile_pool(name="yun", bufs=1) as yun,
            tc.tile_pool(name="lt", bufs=2) as ltp,
            tc.tile_pool(name="psa", bufs=2, space="PSUM") as psa,
            tc.tile_pool(name="pss", bufs=2, space="PSUM") as pss,
            tc.tile_pool(name="psy", bufs=1, space="PSUM") as psy,
            tc.tile_pool(name="dr", bufs=2, space="DRAM") as drp,
        ):
            # ---- per-batch QKV + v-transpose as a resumable step stream ----
            states = {}
            consts = {}

            def make_state(b):
                st = {
                    "qT": qkp.tile([P, T], r32, tag="q", name="qT"),
                    "kp": qkp.tile([P, 2, T], r32, tag="k", name="kp"),
                    "vT": qkp.tile([P, T], f32, tag="v", name="vT"),
                    "yT": ytp.tile([P, T], r32, name="yT"),
                }
                # zero the complementary head rows once; head h lives in its
                # natural psum rows, the other 64 rows stay 0 so K=128
                # matmuls against the full qT tile are exact (and keep the
                # PE activity monitor at full clock - K=64 never warms it)
                nc.vector.tensor_copy(
                    st["kp"][64:128, 0, :],
                    consts["zero64"][:, 0:1].to_broadcast([64, T]),
                )
                nc.vector.tensor_copy(
                    st["kp"][0:64, 1, :],
                    consts["zero64"][:, 0:1].to_broadcast([64, T]),
                )
                bt0 = b * T
                dsts = [st["qT"], None, st["vT"]]
                x_rs = {}

                def load_chunk(n):
                    x_f = stage.tile([P, 8, XC], f32, tag="stage")
                    t0 = bt0 + n * XC
                    nc.sync.dma_start(x_f[:], xt_ap[:, :, t0 : t0 + XC])
                    x_r = xrp.tile([P, 8, XC], r32)
                    nc.gpsimd.tensor_copy(x_r[:], x_f[:])
                    x_rs[n] = x_r

                def gen():
                    load_chunk(0)
                    yield
                    load_chunk(1)
                    yield
                    for n in range(NCH):
                        for m in range(3):
                            ps = psa.tile([P, XC], f32, tag="a")
                            for ko in range(8):
                                nc.tensor.matmul(
                                    ps[:],
                                    consts["wq_r"][:, ko, m * P : (m + 1) * P],
                                    x_rs[n][:, ko, :],
                                    start=(ko == 0),
                                    stop=(ko == 7),
                                )
                            if m == 1:
                                kp = st["kp"]
                                nc.vector.tensor_scalar_add(
                                    kp[0:64, 0, n * XC : (n + 1) * XC],
                                    ps[0:64, :],
                                    consts["b_sb"][0:64, 1:2],
                                )
                                nc.vector.tensor_scalar_add(
                                    kp[64:128, 1, n * XC : (n + 1) * XC],
                                    ps[64:128, :],
                                    consts["b_sb"][64:128, 1:2],
                                )
                            else:
                                nc.vector.tensor_scalar_add(
                                    dsts[m][:, n * XC : (n + 1) * XC],
                                    ps[:],
                                    consts["b_sb"][:, m : m + 1],
                                )
                            yield
                        del x_rs[n]
                        if n + 2 < NCH:
                            load_chunk(n + 2)
                            yield

                    v_aug = vap.tile([P, T // P, 2, DH + 1], r32)
                    st["v_aug"] = v_aug
                    nc.vector.tensor_copy(
                        v_aug[:, :, :, DH : DH + 1], consts["ones_col"][:]
                    )
                    for t in range(T // P):
                        pt = psa.tile([P, P], f32, tag="a")
                        nc.tensor.transpose(
                            pt[:], st["vT"][:, t * P : (t + 1) * P],
                            consts["ident"][:],
                        )
                        nc.vector.tensor_copy(
                            v_aug[:, t, :, 0:DH],
                            pt[:].rearrange("p (h d) -> p h d", h=2),
                        )
                        yield

                st["gen"] = gen()
                return st

            def get_state(b):
                if b not in states:
                    states[b] = make_state(b)
                return states[b]

            def filler(b):
                """Emit one next-batch QKV/transpose step as PE filler."""
                if b < B:
                    next(get_state(b)["gen"], None)

            # ---- constants / weights ----
            ident = cst.tile([P, P], f32)
            make_identity(nc, ident[:])
            consts["ident"] = ident

            # transposed causal mask: mask[p, j] = 0 if j >= p else -1e9
            maskT = cst.tile([P, P], f32)
            nc.gpsimd.memset(maskT[:], 0.0)
            nc.gpsimd.affine_select(
                out=maskT[:],
                in_=maskT[:],
                compare_op=mybir.AluOpType.is_ge,
                fill=-1e9,
                base=0,
                pattern=[[1, P]],
                channel_multiplier=-1,
            )

            ones_f = cst.tile([1, DH], f32)
            nc.vector.memset(ones_f[:], 1.0)
            ones_r = cst.tile([1, DH], r32)
            nc.vector.tensor_copy(ones_r[:], ones_f[:])

            ones_col = cst.tile([P, T // P, 2, 1], f32)
            nc.vector.memset(ones_col[:], 1.0)
            consts["ones_col"] = ones_col

            zero64 = cst.tile([64, 1], f32)
            nc.vector.memset(zero64[:], 0.0)
            consts["zero64"] = zero64

            wq_f = cst.tile([P, 8, 3 * CPC], f32, tag="wstage", name="wq_f")
            nc.gpsimd.dma_start(wq_f[:], wq_ap[:])
            wq_r = cst.tile([P, 8, 3 * CPC], r32)
            nc.vector.tensor_copy(wq_r[:], wq_f[:])
            consts["wq_r"] = wq_r

            wp_f = cst.tile([CPC, C], f32, tag="wstage", name="wp_f")
            nc.gpsimd.dma_start(wp_f[:], w_proj_c[:])
            wp_r = cst.tile([CPC, C], r32)
            nc.vector.tensor_copy(wp_r[:], wp_f[:])

            b_sb = cst.tile([P, 3], f32)
            nc.gpsimd.dma_start(b_sb[:], b_qkv_c[:])
            consts["b_sb"] = b_sb

            # deferred PE-side epilogue tail of the previous attention group
            pending_late = [None]

            def pump_late():
                if pending_late[0] is not None:
                    pending_late[0]()
                    pending_late[0] = None

            for b in range(B):
                bt0 = b * T
                st = get_state(b)
                for _ in st["gen"]:
                    pass
                qT, kp, v_aug, yT = st["qT"], st["kp"], st["v_aug"], st["yT"]

                # ---- attention ----
                for h in range(2):
                    h0 = h * DH
                    for qg in range(T // 1024):
                        q0 = qg * 1024
                        ktmax = 8 * qg + 8
                        ps_y = psy.tile([P, 1024], f32)

                        def emit_av(kt, es, off, ps_y=ps_y, h=h, ktmax=ktmax,
                                    v_aug=v_aug):
                            for c0, c1 in segs(off):
                                nc.tensor.matmul(
                                    ps_y[0 : DH + 1, c0:c1],
                                    v_aug[:, kt, h, :],
                                    es[:, c0:c1],
                                    start=(kt == 0),
                                    stop=(kt == ktmax - 1),
                                    skip_group_check=True,
                                )

                        pending_av = None
                        for kt in range(ktmax):
                            d = kt - 8 * qg
                            off = max(0, d * P)
                            ps_s = pss.tile([P, 1024], f32, tag="s")
                            for c0, c1 in segs(off):
                                nc.tensor.matmul(
                                    ps_s[:, c0:c1],
                                    kp[:, h, kt * P : (kt + 1) * P],
                                    qT[:, q0 + c0 : q0 + c1],
                                    start=True,
                                    stop=True,
                                )
                            if d >= 0:
                                nc.vector.tensor_tensor(
                                    ps_s[:, off : off + P],
                                    ps_s[:, off : off + P],
                                    maskT[:],
                                    ADD,
                                )
                            es = esp.tile([P, 1024], r32)
                            nc.scalar.activation(
                                es[:, off:1024], ps_s[:, off:1024], Exp,
                                scale=0.125,
                            )
                            if kt in (1, 4, 7, 10, 13):
                                filler(b + 1)
                            if pending_av is not None:
                                emit_av(*pending_av)
                            pending_av = (kt, es, off)
                            if kt == 2:
                                pump_late()
                        emit_av(*pending_av)

                        # early epilogue: free psY, compute r = 1/l via a
                        # DRAM-bounce reshape (reciprocal on 128 partitions)
                        y_un = yun.tile([DH + 1, 1024], f32)
                        nc.vector.tensor_copy(y_un[:], ps_y[0 : DH + 1, :])
                        l_dram = drp.tile([1024], f32, tag="ld")
                        nc.sync.dma_start(l_dram[:], y_un[DH : DH + 1, :])
                        l_t = ltp.tile([P, 8], f32, tag="lt")
                        nc.sync.dma_start(
                            l_t[:], l_dram.rearrange("(p f) -> p f", p=P)
                        )
                        r_t = ltp.tile([P, 8], f32, tag="rt")
                        nc.vector.reciprocal(r_t[:], l_t[:])
                        r_dram = drp.tile([1024], f32, tag="rd")
                        nc.sync.dma_start(
                            r_dram.rearrange("(p f) -> p f", p=P), r_t[:]
                        )
                        r_f = rrp.tile([1, 1024], f32, tag="rf")
                        nc.sync.dma_start(r_f[:], r_dram[:].unsqueeze(0))
                        r_r = rrp.tile([1, 1024], r32, tag="rr")
                        nc.vector.tensor_copy(r_r[:], r_f[:])

                        def late(y_un=y_un, r_r=r_r, h0=h0, q0=q0, yT=yT):
                            for half in (0, 1):
                                c0 = half * 512
                                ps_b = psa.tile([P, 512], f32, tag="a")
                                nc.tensor.matmul(
                                    ps_b[0:DH, :],
                                    ones_r[:],
                                    r_r[:, c0 : c0 + 512],
                                    start=True,
                                    stop=True,
                                )
                                nc.vector.tensor_tensor(
                                    yT[h0 : h0 + DH, q0 + c0 : q0 + c0 + 512],
                                    y_un[0:DH, c0 : c0 + 512],
                                    ps_b[0:DH, :],
                                    MULT,
                                )

                        pending_late[0] = late

                # ---- proj ----
                for mt in range(T // P):
                    for ng in range(C // 512):
                        ps = psa.tile([P, 512], f32, tag="a")
                        nc.tensor.matmul(
                            ps[:],
                            yT[:, mt * P : (mt + 1) * P],
                            wp_r[:, ng * 512 : (ng + 1) * 512],
                            start=True,
                            stop=True,
                        )
                        o = oop.tile([P, 512], f32)
                        nc.vector.tensor_copy(o[:], ps[:])
                        nc.sync.dma_start(
                            y_part[
                                bt0 + mt * P : bt0 + (mt + 1) * P,
                                ng * 512 : (ng + 1) * 512,
                            ],
                            o[:],
                        )
                    if mt == 7:
                        pump_late()

            pump_late()

    nc.finalize()
    return nc


def kernel(x, w_qkv, b_qkv, w_proj, b_proj):
    global LAST_EXEC_NS
    from concourse.bass_utils import run_bass_kernel_spmd

    x = np.asarray(x, dtype=np.float32)
    w_qkv = np.asarray(w_qkv, dtype=np.float32)
    b_qkv = np.asarray(b_qkv, dtype=np.float32)
    w_proj = np.asarray(w_proj, dtype=np.float32)
    b_proj = np.asarray(b_proj, dtype=np.float32)

    x_t = np.ascontiguousarray(x.reshape(TT, C).T)

    in_maps = []
    for c in range(NCORES):
        s = c * CPC
        wq = np.ascontiguousarray(
            np.concatenate(
                [
                    w_qkv[:, s : s + CPC],
                    w_qkv[:, C + s : C + s + CPC],
                    w_qkv[:, 2 * C + s : 2 * C + s + CPC],
                ],
                axis=1,
            )
        )
        bq = np.ascontiguousarray(
            np.stack(
                [
                    b_qkv[s : s + CPC],
                    b_qkv[C + s : C + s + CPC],
                    b_qkv[2 * C + s : 2 * C + s + CPC],
                ],
                axis=1,
            )
        )
        wp = np.ascontiguousarray(w_proj[s : s + CPC, :])
        in_maps.append(
            {"x_t": x_t, "w_qkv_c": wq, "b_qkv_c": bq, "w_proj_c": wp}
        )

    if not _NC_CACHE:
        _NC_CACHE.append(_build())
    nc = _NC_CACHE[0]

    res = run_bass_kernel_spmd(
        nc, in_maps, list(range(NCORES)), trace=TRACE
    )
    LAST_EXEC_NS = res.exec_time_ns

    out = res.results[0]["y_part"].astype(np.float64)
    for c in range(1, NCORES):
        out += res.results[c]["y_part"]
    out = (out + b_proj).astype(np.float32)
    return out.reshape(B, T, C)


# revision 23
# speedup vs baseline: 1.4631x; 1.4631x over previous
"""Causal self-attention (B=4, T=2048, C=1024, H=16, Dh=64) on 8 TRN2 cores.

Sharding: tensor-parallel over heads. Core c owns heads {2c, 2c+1}:
  - w_qkv column-sliced -> [1024, 384] (128 q-cols | 128 k-cols | 128 v-cols)
  - w_proj row-sliced   -> [128, 1024]
  - x is broadcast to all cores, pre-transposed on host to feature-major
    x_t [1024, B*T] so the contraction dim (C) lands on SBUF partitions.
  - each core emits a partial y [B*T, 1024]; host sums the 8 partials and
    adds b_proj.

Per-core pipeline (all matmuls fp32r = full-rate fp32):
  A. QKV: qkv_T = w_slice.T @ x_t -> q,k feature-major [128, T] fp32r;
     v feature-major f32, then PE-transposed to token-major [Tk, 65] with a
     fused ones column (row 64 of the att@v psum = softmax denominator l).
  B. Attention per (head, 1024-query group), key chunks of 128:
     S^T = k_chunk.T @ q (K=64), causal mask add on the diagonal chunk,
     Exp on ACT (scale=1/8, no max subtraction - scores are O(10)),
     att@v accumulates y^T[65, Tq] in PSUM.
  C. Proj: y_part[tok, :] = y_T.T @ w_proj_slice, written token-major.

Scheduling notes. TRN2 engines execute their queues IN ORDER and the PE
clock halves (HAM) whenever the PE idles more than a few us, so emission
order is arranged to keep the PE stream dense:
  - av matmuls trail the S matmuls by one key chunk (never wait on ACT);
  - QKV is a resumable step stream; next-batch steps are interleaved as PE
    filler at attention group starts. x chunks are 256 tokens, DMA'd two
    chunks ahead, split across the sync+gpsimd queues, and rounded to
    fp32r on the (otherwise idle) GpSimd engine so the DVE queue stays
    clear;
  - softmax normalization is split: one DVE copy frees the psum
    accumulator, l bounces through DRAM into [128, 8] for a parallel
    reciprocal, and the PE tail (K=1 ones-matmul broadcast of r + DVE
    multiply into y_T) is deferred into the next group's kt loop.
"""

import sys

sys.path.insert(0, "/opt/trn_rl_repo")
import numpy as np

B = 4
T = 2048
C = 1024
H = 16
DH = 64
NCORES = 8
CPC = 128  # channels (=2 heads * 64) per core
TT = B * T
P = 128
XC = 256  # x-chunk tokens
NCH = T // XC  # 8 chunks per batch

TRACE = False
LAST_EXEC_NS = None

_NC_CACHE = []


def _build():
    import concourse.bacc as bacc
    import concourse.mybir as mybir
    import concourse.tile as tile
    from concourse.masks import make_identity

    f32 = mybir.dt.float32
    r32 = mybir.dt.float32r
    Exp = mybir.ActivationFunctionType.Exp
    ADD = mybir.AluOpType.add
    MULT = mybir.AluOpType.mult

    nc = bacc.Bacc(target_bir_lowering=False)

    x_t = nc.dram_tensor("x_t", [C, TT], r32, kind="ExternalInput")
    w_qkv_c = nc.dram_tensor("w_qkv_c", [C, 3 * CPC], r32, kind="ExternalInput")
    b_qkv_c = nc.dram_tensor("b_qkv_c", [P, 3], f32, kind="ExternalInput")
    w_proj_c = nc.dram_tensor("w_proj_c", [CPC, C], r32, kind="ExternalInput")
    y_part = nc.dram_tensor("y_part", [TT, C], f32, kind="ExternalOutput")

    xt_ap = x_t.rearrange("(ko p) t -> p ko t", p=P)  # [128, 8, TT]
    wq_ap = w_qkv_c.rearrange("(ko p) f -> p ko f", p=P)  # [128, 8, 384]

    def segs(off):
        """Split [off, 1024) at the 512 psum-bank boundary."""
        if off < 512:
            return [(off, 512), (512, 1024)]
        if off < 1024:
            return [(off, 1024)]
        return []

    with tile.TileContext(nc) as tc:
        with (
            tc.tile_pool(name="cst", bufs=1) as cst,
            tc.tile_pool(name="stage", bufs=3) as stage,
            tc.tile_pool(name="qk", bufs=2) as qkp,
            tc.tile_pool(name="va", bufs=2) as vap,
            tc.tile_pool(name="yt", bufs=2) as ytp,
            tc.tile_pool(name="es", bufs=3) as esp,
            tc.tile_pool(name="oo", bufs=3) as oop,
            tc.tile_pool(name="rr", bufs=2) as rrp,
            tc.tile_pool(name="yun", bufs=2) as yun,
            tc.tile_pool(name="lt", bufs=2) as ltp,
            tc.tile_pool(name="psa", bufs=2, space="PSUM") as psa,
            tc.tile_pool(name="pss", bufs=2, space="PSUM") as pss,
            tc.tile_pool(name="psy", bufs=1, space="PSUM") as psy,
            tc.tile_pool(name="dr", bufs=2, space="DRAM") as drp,
        ):
            # ---- per-batch QKV + v-transpose as a resumable step stream ----
            states = {}
            consts = {}

            def make_state(b):
                st = {
                    "qT": qkp.tile([P, T], r32, tag="q", name="qT"),
                    "kp": qkp.tile([P, 2, T], r32, tag="k", name="kp"),
                    "vT": qkp.tile([P, T], f32, tag="v", name="vT"),
                    "yT": ytp.tile([P, T], r32, name="yT"),
                }
                # zero the complementary head rows once; head h lives in its
                # natural psum rows, the other 64 rows stay 0 so K=128
                # matmuls against the full qT tile are exact (and keep the
                # PE activity monitor at full clock - K=64 never warms it)
                nc.vector.tensor_copy(
                    st["kp"][64:128, 0, :],
                    consts["zero64"][:, 0:1].to_broadcast([64, T]),
                )
                nc.vector.tensor_copy(
                    st["kp"][0:64, 1, :],
                    consts["zero64"][:, 0:1].to_broadcast([64, T]),
                )
                bt0 = b * T
                dsts = [st["qT"], None, st["vT"]]
                x_rs = {}

                def load_chunk(n):
                    x_r = stage.tile([P, 8, XC], r32, tag="stage", name="x_r")
                    t0 = bt0 + n * XC
                    nc.sync.dma_start(x_r[:], xt_ap[:, :, t0 : t0 + XC])
                    x_rs[n] = x_r

                def gen():
                    load_chunk(0)
                    yield
                    load_chunk(1)
                    yield
                    for n in range(NCH):
                        for m in range(3):
                            ps = psa.tile([P, XC], f32, tag="a")
                            for ko in range(8):
                                nc.tensor.matmul(
                                    ps[:],
                                    consts["wq_r"][:, ko, m * P : (m + 1) * P],
                                    x_rs[n][:, ko, :],
                                    start=(ko == 0),
                                    stop=(ko == 7),
                                )
                            if m == 1:
                                kp = st["kp"]
                                nc.vector.tensor_scalar_add(
                                    kp[0:64, 0, n * XC : (n + 1) * XC],
                                    ps[0:64, :],
                                    consts["b_sb"][0:64, 1:2],
                                )
                                nc.vector.tensor_scalar_add(
                                    kp[64:128, 1, n * XC : (n + 1) * XC],
                                    ps[64:128, :],
                                    consts["b_sb"][64:128, 1:2],
                                )
                            else:
                                nc.vector.tensor_scalar_add(
                                    dsts[m][:, n * XC : (n + 1) * XC],
                                    ps[:],
                                    consts["b_sb"][:, m : m + 1],
                                )
                            yield
                        del x_rs[n]
                        if n + 2 < NCH:
                            load_chunk(n + 2)
                            yield

                    v_aug = vap.tile([P, T // P, 2, DH + 1], r32)
                    st["v_aug"] = v_aug
                    nc.vector.tensor_copy(
                        v_aug[:, :, :, DH : DH + 1], consts["ones_col"][:]
                    )
                    for t in range(T // P):
                        pt = psa.tile([P, P], f32, tag="a")
                        nc.tensor.transpose(
                            pt[:], st["vT"][:, t * P : (t + 1) * P],
                            consts["ident"][:],
                        )
                        nc.vector.tensor_copy(
                            v_aug[:, t, :, 0:DH],
                            pt[:].rearrange("p (h d) -> p h d", h=2),
                        )
                        yield

                st["gen"] = gen()
                return st

            def get_state(b):
                if b not in states:
                    states[b] = make_state(b)
                return states[b]

            def filler(b):
                """Emit one next-batch QKV/transpose step as PE filler."""
                if b < B:
                    next(get_state(b)["gen"], None)

            # ---- constants / weights ----
            ident = cst.tile([P, P], f32)
            make_identity(nc, ident[:])
            consts["ident"] = ident

            # transposed causal mask: mask[p, j] = 0 if j >= p else -1e9
            maskT = cst.tile([P, P], f32)
            nc.gpsimd.memset(maskT[:], 0.0)
            nc.gpsimd.affine_select(
                out=maskT[:],
                in_=maskT[:],
                compare_op=mybir.AluOpType.is_ge,
                fill=-1e9,
                base=0,
                pattern=[[1, P]],
                channel_multiplier=-1,
            )

            ones_f = cst.tile([1, DH], f32)
            nc.vector.memset(ones_f[:], 1.0)
            ones_r = cst.tile([1, DH], r32)
            nc.vector.tensor_copy(ones_r[:], ones_f[:])

            ones_col = cst.tile([P, T // P, 2, 1], f32)
            nc.vector.memset(ones_col[:], 1.0)
            consts["ones_col"] = ones_col

            zero64 = cst.tile([64, 1], f32)
            nc.vector.memset(zero64[:], 0.0)
            consts["zero64"] = zero64

            wq_r = cst.tile([P, 8, 3 * CPC], r32)
            nc.gpsimd.dma_start(wq_r[:], wq_ap[:])
            consts["wq_r"] = wq_r

            wp_r = cst.tile([CPC, C], r32)
            nc.gpsimd.dma_start(wp_r[:], w_proj_c[:])

            b_sb = cst.tile([P, 3], f32)
            nc.gpsimd.dma_start(b_sb[:], b_qkv_c[:])
            consts["b_sb"] = b_sb

            # deferred PE-side epilogue tail of the previous attention group
            pending_late = [None]

            def pump_late():
                if pending_late[0] is not None:
                    pending_late[0]()
                    pending_late[0] = None

            for b in range(B):
                bt0 = b * T
                st = get_state(b)
                for _ in st["gen"]:
                    pass
                qT, kp, v_aug, yT = st["qT"], st["kp"], st["v_aug"], st["yT"]

                # ---- attention ----
                for h in range(2):
                    h0 = h * DH
                    for qg in range(T // 1024):
                        q0 = qg * 1024
                        ktmax = 8 * qg + 8
                        ps_y = psy.tile([P, 1024], f32)

                        def emit_av(kt, es, off, ps_y=ps_y, h=h, ktmax=ktmax,
                                    v_aug=v_aug):
                            for c0, c1 in segs(off):
                                nc.tensor.matmul(
                                    ps_y[0 : DH + 1, c0:c1],
                                    v_aug[:, kt, h, :],
                                    es[:, c0:c1],
                                    start=(kt == 0),
                                    stop=(kt == ktmax - 1),
                                    skip_group_check=True,
                                )

                        pending_av = None
                        for kt in range(ktmax):
                            d = kt - 8 * qg
                            off = max(0, d * P)
                            ps_s = pss.tile([P, 1024], f32, tag="s")
                            for c0, c1 in segs(off):
                                nc.tensor.matmul(
                                    ps_s[:, c0:c1],
                                    kp[:, h, kt * P : (kt + 1) * P],
                                    qT[:, q0 + c0 : q0 + c1],
                                    start=True,
                                    stop=True,
                                )
                            if d >= 0:
                                nc.vector.tensor_tensor(
                                    ps_s[:, off : off + P],
                                    ps_s[:, off : off + P],
                                    maskT[:],
                                    ADD,
                                )
                            es = esp.tile([P, 1024], r32)
                            nc.scalar.activation(
                                es[:, off:1024], ps_s[:, off:1024], Exp,
                                scale=0.125,
                            )
                            if kt in (1, 4, 7, 10, 13):
                                filler(b + 1)
                            if pending_av is not None:
                                emit_av(*pending_av)
                            pending_av = (kt, es, off)
                            if kt == 2:
                                pump_late()
                        emit_av(*pending_av)

                        # early epilogue: free psY, compute r = 1/l via a
                        # DRAM-bounce reshape (reciprocal on 128 partitions)
                        y_un = yun.tile([DH + 1, 1024], f32)
                        nc.vector.tensor_copy(y_un[:], ps_y[0 : DH + 1, :])
                        l_dram = drp.tile([1024], f32, tag="ld")
                        nc.sync.dma_start(l_dram[:], y_un[DH : DH + 1, :])
                        l_t = ltp.tile([P, 8], f32, tag="lt")
                        nc.sync.dma_start(
                            l_t[:], l_dram.rearrange("(p f) -> p f", p=P)
                        )
                        r_t = ltp.tile([P, 8], f32, tag="rt")
                        nc.vector.reciprocal(r_t[:], l_t[:])
                        r_dram = drp.tile([1024], f32, tag="rd")
                        nc.sync.dma_start(
                            r_dram.rearrange("(p f) -> p f", p=P), r_t[:]
                        )
                        r_r = rrp.tile([1, 1024], r32, tag="rr")
                        nc.sync.dma_start(
                            r_r[:], r_dram[:].bitcast(r32).unsqueeze(0)
                        )

                        def late(y_un=y_un, r_r=r_r, h0=h0, q0=q0, yT=yT):
                            for half in (0, 1):
                                c0 = half * 512
                                ps_b = psa.tile([P, 512], f32, tag="a")
                                nc.tensor.matmul(
                                    ps_b[0:DH, :],
                                    ones_r[:],
                                    r_r[:, c0 : c0 + 512],
                                    start=True,
                                    stop=True,
                                )
                                nc.vector.tensor_tensor(
                                    yT[h0 : h0 + DH, q0 + c0 : q0 + c0 + 512],
                                    y_un[0:DH, c0 : c0 + 512],
                                    ps_b[0:DH, :],
                                    MULT,
                                )

                        pending_late[0] = late

                # ---- proj ----
                for mt in range(T // P):
                    for ng in range(C // 512):
                        ps = psa.tile([P, 512], f32, tag="a")
                        nc.tensor.matmul(
                            ps[:],
                            yT[:, mt * P : (mt + 1) * P],
                            wp_r[:, ng * 512 : (ng + 1) * 512],
                            start=True,
                            stop=True,
                        )
                        o = oop.tile([P, 512], f32)
                        nc.vector.tensor_copy(o[:], ps[:])
                        nc.sync.dma_start(
                            y_part[
                                bt0 + mt * P : bt0 + (mt + 1) * P,
                                ng * 512 : (ng + 1) * 512,
                            ],
                            o[:],
                        )
                    if mt == 7:
                        pump_late()

            pump_late()

    nc.finalize()
    return nc


def kernel(x, w_qkv, b_qkv, w_proj, b_proj):
    global LAST_EXEC_NS
    from concourse.bass_utils import run_bass_kernel_spmd

    x = np.asarray(x, dtype=np.float32)
    w_qkv = np.asarray(w_qkv, dtype=np.float32)
    b_qkv = np.asarray(b_qkv, dtype=np.float32)
    w_proj = np.asarray(w_proj, dtype=np.float32)
    b_proj = np.asarray(b_proj, dtype=np.float32)

    x_t = np.ascontiguousarray(x.reshape(TT, C).T)

    in_maps = []
    for c in range(NCORES):
        s = c * CPC
        wq = np.ascontiguousarray(
            np.concatenate(
                [
                    w_qkv[:, s : s + CPC],
                    w_qkv[:, C + s : C + s + CPC],
                    w_qkv[:, 2 * C + s : 2 * C + s + CPC],
                ],
                axis=1,
            )
        )
        bq = np.ascontiguousarray(
            np.stack(
                [
                    b_qkv[s : s + CPC],
                    b_qkv[C + s : C + s + CPC],
                    b_qkv[2 * C + s : 2 * C + s + CPC],
                ],
                axis=1,
            )
        )
        wp = np.ascontiguousarray(w_proj[s : s + CPC, :])
        in_maps.append(
            {"x_t": x_t, "w_qkv_c": wq, "b_qkv_c": bq, "w_proj_c": wp}
        )

    if not _NC_CACHE:
        _NC_CACHE.append(_build())
    nc = _NC_CACHE[0]

    res = run_bass_kernel_spmd(
        nc, in_maps, list(range(NCORES)), trace=TRACE
    )
    LAST_EXEC_NS = res.exec_time_ns

    out = res.results[0]["y_part"].astype(np.float64)
    for c in range(1, NCORES):
        out += res.results[c]["y_part"]
    out = (out + b_proj).astype(np.float32)
    return out.reshape(B, T, C)
